# revision 1
# baseline (speedup 1.0000x reference)
"""Trainium2 Bass kernel for nn_AdapterMLP (gnn_message_passing).

Strategy (8 independent NeuronCores, no collectives):
  - Shard (batch=4) x (seq halves=2) -> 8 shards of [1024, 4096] rows.
  - All gather/scatter index structure is resolved on the host into
    dense one-hot matmul operands (A_g for the subtoken mean-pool
    "message passing" gather, S_sel for the last-wins scatter), so the
    device kernel is pure dense compute.
  - Algebraic shortcut: aw[w,e] = <ents_t[w,e,:], b[w,:]> is computed as
    <(g*u)[w,e,:], (b @ down_w)[w,:]>, eliminating the [1152,1024]x
    [1024,4096] per-item down-projection (8x fewer word-branch FLOPs).
  - The scatter branch is folded into the big MLP as one extra K-tile:
    pre = scale_s*(x @ Wh'^T) + [tmpT | 1] @ [Wt^T ; mlp_b], where
    Wh' = mlp_w[:, :D] * ln_weight (RMS scale commutes past the matmul).
  - TensorE runs bf16 (f32 accumulate in PSUM); weights/activations are
    pre-cast/pre-transposed on the host.
"""
import os
import sys

sys.path.insert(0, "/opt/trn_rl_repo")

import numpy as np
from ml_dtypes import bfloat16

import concourse.bass as bass
import concourse.bacc as bacc
import concourse.tile as tile
from concourse import mybir
from concourse.bass_utils import run_bass_kernel_spmd

B, S, D = 4, 2048, 4096
W, E, T = 128, 8, 4
KD, KI = 100, 1024
EPS = 1e-06
NCORES = 8
SL = S // 2        # 1024 rows per core
GR = 512           # gathered rows per core (W*T upper bound)
P = 128
FB = 512           # psum free dim
NK = D // P        # 32 k-tiles
NN = D // FB       # 8 n-chunks
NM = SL // P       # 8 m-tiles
NE = E + 1         # 9

f32 = mybir.dt.float32
bf = mybir.dt.bfloat16
AF = mybir.ActivationFunctionType
ALU = mybir.AluOpType
AX = mybir.AxisListType


def _bf(a):
    return np.ascontiguousarray(a.astype(bfloat16))


def build_core_inputs(inp, core):
    """Host preprocessing for one core: slice/transpose/cast + index->matrix."""
    b, h = core // 2, core % 2
    r0 = h * SL
    x = np.asarray(inp["output_hidden_states"], np.float32)
    we_b = np.asarray(inp["words_ents"])[b]
    ws_b = np.asarray(inp["words_subtoken"])[b]
    ce = np.asarray(inp["concept_embed"], np.float32)
    sent = np.asarray(inp["sentinel"], np.float32).reshape(KD)
    lnw = np.asarray(inp["ln_weight"], np.float32)
    gw = np.asarray(inp["gate_w"], np.float32)
    uw = np.asarray(inp["up_w"], np.float32)
    dw = np.asarray(inp["down_w"], np.float32)
    mw = np.asarray(inp["mlp_w"], np.float32)
    mb = np.asarray(inp["mlp_b"], np.float32)
    alpha = float(np.asarray(inp["alpha"]).reshape(-1)[0])

    xl = x[b, r0:r0 + SL]                                    # [SL, D]
    xt = np.ascontiguousarray(xl.T)                          # [D, SL]

    # b-gather rows: unique subtoken indices of this item (pad index S dropped)
    idxf = np.where(ws_b == -1, S, ws_b)                     # [W,T]
    flat = idxf.reshape(-1)
    uniq = np.unique(flat[flat < S])
    gidx = np.zeros(GR, np.int64)
    gidx[:uniq.size] = uniq
    xg = x[b, gidx]                                          # [GR, D]
    cnt = np.maximum(np.sum(ws_b != -1, axis=1), 1).astype(np.float32)
    ag = np.zeros((GR, W), np.float32)
    pos = {int(s_): i for i, s_ in enumerate(uniq)}
    for w in range(W):
        for t in range(T):
            s_ = int(idxf[w, t])
            if s_ < S:
                ag[pos[s_], w] += 1.0 / cnt[w]

    # entity embeddings (host gather of the concept table)
    we_idx = np.where(we_b == -1, 0, we_b)
    ents = ce[we_idx]                                        # [W,E,KD]
    ent_ori = np.concatenate(
        [ents, np.broadcast_to(sent.reshape(1, 1, KD), (W, 1, KD))], axis=1)
    entw = np.ascontiguousarray(ent_ori.transpose(1, 0, 2))  # [NE, W, KD]
    entt = np.zeros((P, NE * W), np.float32)                 # KD padded to 128
    entt[:KD] = entw.reshape(NE * W, KD).T

    # scatter one-hot: winner = last (w,t) in flat order; local half only
    sst = np.zeros((W, SL), np.float32)
    winner = {}
    for w in range(W):
        for t in range(T):
            s_ = int(idxf[w, t])
            if s_ < S:
                winner[s_] = w
    for s_, w in winner.items():
        if r0 <= s_ < r0 + SL:
            sst[w, s_ - r0] = 1.0

    # weights: fold ln into Wh and down_w; pre-transpose; tile wk for DMA
    whT = (mw[:, :D] * lnw[None, :]).T                       # [D, D]
    wtT = mw[:, D:].T                                        # [KD, D]
    wk = np.zeros((NN, NK + 1, P, FB), np.float32)
    for n in range(NN):
        cs = slice(n * FB, (n + 1) * FB)
        for k in range(NK):
            wk[n, k] = whT[k * P:(k + 1) * P, cs]
        wk[n, NK, :KD] = wtT[:, cs]
        wk[n, NK, KD] = mb[cs]
    dwt = dw * lnw[:, None]                                  # [D, KI]

    mask = np.where(
        np.concatenate([we_b, np.ones((W, 1), we_b.dtype)], -1) == -1,
        -1e9, 0.0).astype(np.float32)

    aux_init = np.zeros((P, SL), np.float32)
    aux_init[KD] = 1.0

    # batch 4 k-tiles per DMA: [G, 128, 4*inner] contiguous blocks
    xt_big = xt.reshape(NK, P, SL).reshape(8, 4, P, SL).transpose(0, 2, 1, 3).reshape(8, P, 4 * SL)
    wk_big = wk[:, :NK].reshape(NN, 8, 4, P, FB).transpose(0, 1, 3, 2, 4).reshape(NN, 8, P, 4 * FB)
    wk_aux = np.ascontiguousarray(wk[:, NK])
    dwt_big = dwt.reshape(NK, P, KI).reshape(16, 2, P, KI).transpose(0, 2, 1, 3).reshape(16, P, 2 * KI)
    return dict(
        xt=_bf(xt_big),
        xrow=np.ascontiguousarray(xl),
        xrow_bf=_bf(xl),
        wk=_bf(wk_big),
        wk_aux=_bf(wk_aux),
        dwt=_bf(dwt_big),
        xg=_bf(xg).reshape(4, P, D),
        ag=_bf(ag).reshape(4, P, W),
        entw=_bf(entw),
        entt=_bf(entt),
        gwt=_bf(np.concatenate([gw.T, np.zeros((P - KD, KI), np.float32)], 0)),
        uwt=_bf(np.concatenate([uw.T, np.zeros((P - KD, KI), np.float32)], 0)),
        sst=_bf(sst),
        mask=np.ascontiguousarray(mask),
        alpha_b=np.full((P, 1), alpha, np.float32),
        aux_init=_bf(aux_init),
    )


def _kernel_body(nc, tc, I, out_ap):
    with tc.tile_pool(name="res", bufs=1) as res, \
         tc.tile_pool(name="small", bufs=1) as small:
        # ======== sync-queue DMAs in priority order ========
        xt_big = []
        for g in range(8):
            t = res.tile([P, 4 * SL], bf, tag=f"xt{g}")
            nc.sync.dma_start(out=t[:], in_=I["xt"][g])
            xt_big.append(t)

        def xt_sl(k, m):
            return xt_big[k // 4][:, (k % 4) * SL + m * P:(k % 4) * SL + (m + 1) * P]

        with tc.tile_pool(name="wk0p", bufs=1) as wk0p, \
             tc.tile_pool(name="mpsum", bufs=1, space="PSUM") as mps:
            wp = tc.alloc_tile_pool(name="word", bufs=1)
            # word inputs (sync queue, right after xt)
            xg_tiles = []
            for g in range(4):
                xg_t = wp.tile([P, D], bf, tag=f"xg{g}", name=f"xg{g}")
                nc.sync.dma_start(out=xg_t[:], in_=I["xg"][g])
                xg_tiles.append(xg_t)
            agm = wp.tile([P, 4 * W], bf, tag="agm")
            for g in range(4):
                nc.sync.dma_start(out=agm[:, g * W:(g + 1) * W], in_=I["ag"][g])
            entt_t = wp.tile([P, NE * W], bf, tag="entt")
            nc.sync.dma_start(out=entt_t[:], in_=I["entt"][:])
            gwt_t = wp.tile([P, KI], bf, tag="gwt")
            nc.sync.dma_start(out=gwt_t[:], in_=I["gwt"][:])
            uwt_t = wp.tile([P, KI], bf, tag="uwt")
            nc.sync.dma_start(out=uwt_t[:], in_=I["uwt"][:])
            sst_t = wp.tile([P, SL], bf, tag="sst")
            nc.sync.dma_start(out=sst_t[:], in_=I["sst"][:])
            dwt_tiles = []
            for kb in range(16):
                dwt_t = wp.tile([P, 2 * KI], bf, tag=f"dwt{kb % 2}",
                                name=f"dwt{kb}")
                nc.sync.dma_start(out=dwt_t[:], in_=I["dwt"][kb])
                dwt_tiles.append(dwt_t)

            # var-pass input (sync queue, after dwt)
            mv = tc.alloc_tile_pool(name="mv", bufs=1)
            if True:
                xr_tiles = []
                for m in range(NM):
                    xr_t = mv.tile([P, D], bf, tag="xr", bufs=2, name=f"xr{m}")
                    nc.sync.dma_start(out=xr_t[:],
                                      in_=I["xrow_bf"][m * P:(m + 1) * P, :])
                    xr_tiles.append(xr_t)

                # ======== scalar-queue DMAs (small) ========
                aux_t = res.tile([P, SL], bf, tag="aux")
                nc.scalar.dma_start(out=aux_t[:], in_=I["aux_init"][:])
                alpha_t = small.tile([P, 1], f32, tag="alpha")
                nc.scalar.dma_start(out=alpha_t[:], in_=I["alpha_b"][:])
                mask_t = small.tile([P, NE], f32, tag="mask")
                nc.scalar.dma_start(out=mask_t[:], in_=I["mask"][:])
                ent_t = wp.tile([P, NE * KD], bf, tag="entw")
                for e in range(NE):
                    nc.scalar.dma_start(out=ent_t[:, e * KD:(e + 1) * KD],
                                        in_=I["entw"][e])

                # ======== gpsimd queue: wk chunk 0 ========
                wk_cache = {}
                grp0 = []
                for j in range(8):
                    wt = wk0p.tile([P, 4 * FB], bf, tag=f"wk0g{j}", name=f"wk0g{j}")
                    nc.gpsimd.dma_start(out=wt[:], in_=I["wk"][0, j])
                    grp0.append(wt)
                at0 = wk0p.tile([P, FB], bf, tag="wk0aux")
                nc.gpsimd.dma_start(out=at0[:], in_=I["wk_aux"][0])
                wk_cache[0] = (grp0, at0)

                scal_t = small.tile([P, NM], f32, tag="scal")
                eps_t = small.tile([P, 1], f32, tag="eps")
                nc.vector.memset(eps_t[:], EPS)

                # ======== word branch compute ========
                pms03 = []

                def emit_pm0(m):
                    pm = mps.tile([P, FB], f32, tag="pm", bufs=6,
                                  name=f"pm0_{m}")
                    for k in range(NK):
                        nc.tensor.matmul(
                            pm[:], lhsT=xt_sl(k, m),
                            rhs=grp0[k // 4][:, (k % 4) * FB:(k % 4 + 1) * FB],
                            start=(k == 0), stop=False)
                    pms03.append(pm)

                with tc.tile_pool(name="wpsum", bufs=2, space="PSUM") as wps:
                    # ACT: squares of gathered rows -> rc (gates ags on DVE)
                    rcs = []
                    for g in range(4):
                        sq = mv.tile([P, D], bf, tag="sqscr", bufs=1, name=f"sqw{g}")
                        var = small.tile([P, 1], f32, tag=f"varg{g}")
                        nc.scalar.activation(sq[:], xg_tiles[g][:], AF.Square,
                                             accum_out=var[:])
                        sd = small.tile([P, 1], f32, tag=f"sdg{g}")
                        nc.scalar.activation(sd[:], var[:], AF.Sqrt, bias=eps_t[:],
                                             scale=1.0 / D)
                        rc = small.tile([P, 1], f32, tag=f"rcg{g}")
                        nc.vector.reciprocal(rc[:], sd[:])
                        rcs.append(rc)
                    ags = wp.tile([P, 4 * W], bf, tag="ags")
                    for g in range(4):
                        nc.vector.tensor_scalar_mul(ags[:, g * W:(g + 1) * W],
                                                    agm[:, g * W:(g + 1) * W],
                                                    rcs[g][:])

                    # DVE: var of main rows (stt with accum), before gu chain
                    varms = []
                    for m in range(NM):
                        vscr = mv.tile([P, D], bf, tag="sqscr", bufs=1, name=f"vscr{m}")
                        varm = small.tile([P, 1], f32, tag=f"varm{m}")
                        nc.vector.scalar_tensor_tensor(
                            out=vscr[:], in0=xr_tiles[m][:], scalar=1.0,
                            in1=xr_tiles[m][:], op0=ALU.mult, op1=ALU.mult,
                            accum_out=varm[:])
                        varms.append(varm)
                    mv.release()

                    # PE: bT
                    bt_tiles = []
                    for dk in range(NK):
                        ps = wps.tile([P, W], f32, tag="wps", name=f"btps{dk}")
                        for g in range(4):
                            nc.tensor.matmul(ps[:],
                                             lhsT=xg_tiles[g][:, dk * P:(dk + 1) * P],
                                             rhs=ags[:, g * W:(g + 1) * W],
                                             start=(g == 0), stop=(g == 3))
                        bt = wp.tile([P, W], bf, tag=f"bt{dk}", name=f"bt{dk}")
                        nc.scalar.copy(bt[:], ps[:])
                        bt_tiles.append(bt)

                    emit_pm0(0)

                    # PE: c = b @ (down_w * lnw)
                    c_bf = wp.tile([P, KI], bf, tag="c")
                    cps = []
                    for i in range(2):
                        cpsi = wps.tile([P, FB], f32, tag="wps", name=f"c_ps{i}")
                        cps.append(cpsi)
                    for kb in range(16):
                        for kk in range(2):
                            dk = kb * 2 + kk
                            for i in range(2):
                                nc.tensor.matmul(
                                    cps[i][:], lhsT=bt_tiles[dk][:],
                                    rhs=dwt_tiles[kb][:, kk * KI + i * FB:kk * KI + (i + 1) * FB],
                                    start=(dk == 0), stop=(dk == NK - 1))
                    for i in range(2):
                        nc.scalar.copy(c_bf[:, i * FB:(i + 1) * FB], cps[i][:])

                    emit_pm0(1)
                    emit_pm0(2)

                    # PE/ACT/DVE: gate/up + gu; aw via stt-accum
                    aw_t = small.tile([P, NE], f32, tag="aw")
                    for e in range(NE):
                        g_sb = wp.tile([P, KI], bf, tag="gsb", bufs=1)
                        u_sb = wp.tile([P, KI], bf, tag="usb", bufs=1)
                        for i in range(2):
                            gp = wps.tile([P, FB], f32, tag="wps", name=f"gp{e}_{i}")
                            nc.tensor.matmul(gp[:], lhsT=entt_t[:, e * P:(e + 1) * P],
                                             rhs=gwt_t[:, i * FB:(i + 1) * FB],
                                             start=True, stop=True)
                            nc.scalar.activation(g_sb[:, i * FB:(i + 1) * FB], gp[:],
                                                 AF.Silu)
                            up = wps.tile([P, FB], f32, tag="wps", name=f"up{e}_{i}")
                            nc.tensor.matmul(up[:], lhsT=entt_t[:, e * P:(e + 1) * P],
                                             rhs=uwt_t[:, i * FB:(i + 1) * FB],
                                             start=True, stop=True)
                            nc.scalar.copy(u_sb[:, i * FB:(i + 1) * FB], up[:])
                        gu = wp.tile([P, KI], bf, tag="gu", bufs=2)
                        nc.vector.tensor_mul(gu[:], g_sb[:], u_sb[:])
                        scr = wp.tile([P, KI], bf, tag="awscr", bufs=1)
                        nc.vector.scalar_tensor_tensor(
                            out=scr[:], in0=gu[:], scalar=1.0, in1=c_bf[:],
                            op0=ALU.mult, op1=ALU.mult,
                            accum_out=aw_t[:, e:e + 1])
                        if e == 4:
                            emit_pm0(3)

                    # ACT: finish per-row scales (before tps copies)
                    sdall_t = small.tile([P, NM], f32, tag="sdall")
                    for m in range(NM):
                        nc.scalar.activation(sdall_t[:, m:m + 1], varms[m][:],
                                         AF.Sqrt, bias=eps_t[:],
                                         scale=1.0 / D)
                        nc.vector.reciprocal(scal_t[:, m:m + 1],
                                         sdall_t[:, m:m + 1])

                    # binv[kd, s] = sd[s]; aux /= scal  (so one psum + ACT-scale works)
                    sd_d = nc.dram_tensor("sd_rt", [P, NM], f32, kind="Internal").ap()
                    nc.scalar.dma_start(out=sd_d[:], in_=sdall_t[:])
                    sd_ff = small.tile([1, SL], f32, tag="sd_ff")
                    nc.scalar.dma_start(
                        out=sd_ff[:].rearrange("a (m p) -> a m p", m=NM),
                        in_=sd_d.rearrange("p m -> m p")[None])
                    sd_fb = small.tile([1, SL], bf, tag="sd_fb")
                    nc.vector.tensor_copy(sd_fb[:], sd_ff[:])
                    ones_r = small.tile([1, P], bf, tag="ones_r")
                    nc.vector.memset(ones_r[:], 1.0)
                    binv = res.tile([P, SL], bf, tag="binv")
                    for i in range(SL // FB):
                        bp = mps.tile([P, FB], f32, tag="pm", bufs=6, name=f"binvp{i}")
                        nc.tensor.matmul(bp[:], lhsT=ones_r[:],
                                                 rhs=sd_fb[:, i * FB:(i + 1) * FB],
                                                 start=True, stop=True)
                        nc.scalar.copy(binv[:, i * FB:(i + 1) * FB], bp[:])

                    if True:
                        # DVE: softmax + attn chain
                        awm = small.tile([P, NE], f32, tag="awm")
                        nc.vector.tensor_add(awm[:], aw_t[:], mask_t[:])
                        mx = small.tile([P, 1], f32, tag="mx")
                        nc.vector.reduce_max(mx[:], awm[:], axis=AX.X)
                        nmx = small.tile([P, 1], f32, tag="nmx")
                        nc.vector.tensor_scalar_mul(nmx[:], mx[:], -1.0)
                        expt = small.tile([P, NE], f32, tag="expt")
                        sume = small.tile([P, 1], f32, tag="sume")
                        nc.scalar.activation(expt[:], awm[:], AF.Exp, bias=nmx[:],
                                             accum_out=sume[:])
                        rse = small.tile([P, 1], f32, tag="rse")
                        nc.vector.reciprocal(rse[:], sume[:])
                        attn = small.tile([P, NE], f32, tag="attn")
                        nc.vector.tensor_scalar_mul(attn[:], expt[:], rse[:])
                        acc_prev = wp.tile([P, KD], f32, tag="acc", bufs=2)
                        nc.vector.tensor_scalar_mul(acc_prev[:], ent_t[:, 0:KD],
                                                    attn[:, 0:1])
                        for e in range(1, NE):
                            acc_new = wp.tile([P, KD], f32, tag="acc", bufs=2,
                                              name=f"acc{e}")
                            nc.vector.scalar_tensor_tensor(
                                out=acc_new[:], in0=ent_t[:, e * KD:(e + 1) * KD],
                                scalar=attn[:, e:e + 1], in1=acc_prev[:],
                                op0=ALU.mult, op1=ALU.add)
                            acc_prev = acc_new
                        ao_pad = wp.tile([P, P], bf, tag="ao_pad")
                        nc.vector.memset(ao_pad[:], 0.0)
                        nc.scalar.copy(ao_pad[:, 0:KD], acc_prev[:])

                        # PE: scatter matmul into aux k-tile
                        for i in range(SL // FB):
                            tps = wps.tile([P, FB], f32, tag="wps", name=f"tps{i}")
                            nc.tensor.matmul(tps[:], lhsT=ao_pad[:],
                                             rhs=sst_t[:, i * FB:(i + 1) * FB],
                                             start=True, stop=True)
                            nc.scalar.copy(aux_t[0:KD, i * FB:(i + 1) * FB],
                                           tps[0:KD, :])
                        nc.vector.tensor_mul(aux_t[:], aux_t[:], binv[:])

            if os.environ.get("K_PROBE"):
                dbg_scal = nc.dram_tensor("dbg_scal", [P, NM], f32, kind="Internal").ap()
                nc.sync.dma_start(out=dbg_scal[:], in_=scal_t[:])
                dbg_aux = nc.dram_tensor("dbg_aux", [P, SL], bf, kind="Internal").ap()
                nc.sync.dma_start(out=dbg_aux[:], in_=aux_t[:])
                dbg_aw = nc.dram_tensor("dbg_aw", [P, NE], f32, kind="Internal").ap()
                nc.sync.dma_start(out=dbg_aw[:], in_=aw_t[:])
                dbg_c = nc.dram_tensor("dbg_c", [P, KI], bf, kind="Internal").ap()
                nc.sync.dma_start(out=dbg_c[:], in_=c_bf[:])

            # ---- word pool closed; epilogue + main loop ----
            wp.release()
            with tc.tile_pool(name="op", bufs=2) as op:
                # ======== epilogue helper ========
                def emit_tail(n, m, pm, wk_aux_t):
                    pa = mps.tile([P, FB], f32, tag="pa", bufs=2,
                                  name=f"pa{n}_{m}")
                    nc.tensor.matmul(pa[:], lhsT=aux_t[:, m * P:(m + 1) * P],
                                     rhs=wk_aux_t[:], start=True, stop=True)
                    pa_sb = op.tile([P, FB], f32, tag="pasb", bufs=2,
                                    name=f"pasb{n}_{m}")
                    nc.scalar.copy(pa_sb[:], pa[:])
                    pre = op.tile([P, FB], f32, tag="pre", bufs=2,
                                  name=f"pre{n}_{m}")
                    nc.vector.scalar_tensor_tensor(
                        out=pre[:], in0=pm[:], scalar=scal_t[:, m:m + 1],
                        in1=pa_sb[:], op0=ALU.mult, op1=ALU.add)
                    nc.scalar.activation(pre[:], pre[:], AF.Silu)
                    xr_c = op.tile([P, FB], f32, tag="xrc", bufs=2,
                                   name=f"xrc{n}_{m}")
                    nc.sync.dma_start(
                        out=xr_c[:],
                        in_=I["xrow"][m * P:(m + 1) * P, n * FB:(n + 1) * FB])
                    nc.vector.scalar_tensor_tensor(
                        out=xr_c[:], in0=pre[:], scalar=alpha_t[:],
                        in1=xr_c[:], op0=ALU.mult, op1=ALU.add)
                    nc.sync.dma_start(
                        out=out_ap[m * P:(m + 1) * P, n * FB:(n + 1) * FB],
                        in_=xr_c[:])

                # ======== main loop ========
                with tc.tile_pool(name="wkp", bufs=1) as wkp:
                    def wk_chunk(n):
                        if n in wk_cache:
                            return wk_cache[n]
                        grp = []
                        for j in range(8):
                            wt = wkp.tile([P, 4 * FB], bf, tag=f"wkg{j}",
                                          bufs=2, name=f"wk{n}g{j}")
                            nc.gpsimd.dma_start(out=wt[:], in_=I["wk"][n, j])
                            grp.append(wt)
                        at = wkp.tile([P, FB], bf, tag="wk_aux", bufs=2,
                                      name=f"wka{n}")
                        nc.gpsimd.dma_start(out=at[:], in_=I["wk_aux"][n])
                        wk_cache[n] = (grp, at)
                        return wk_cache[n]

                    wk_chunk(1)
                    wk_chunk(2)
                    for n in range(NN):
                        wk_grp, wk_aux_t = wk_chunk(n)
                        for m in range(NM):
                            if n == 0 and m < 4:
                                pm = pms03[m]
                            else:
                                pm = mps.tile([P, FB], f32, tag="pm",
                                              bufs=6, name=f"pm{n}_{m}")
                                for k in range(NK):
                                    nc.tensor.matmul(
                                        pm[:], lhsT=xt_sl(k, m),
                                        rhs=wk_grp[k // 4][:, (k % 4) * FB:(k % 4 + 1) * FB],
                                        start=(k == 0), stop=False)
                            nc.tensor.matmul(pm[:], lhsT=aux_t[:, m * P:(m + 1) * P],
                                             rhs=wk_aux_t[:], start=False, stop=True)
                            pre_sb = op.tile([P, FB], f32, tag="pre", bufs=3,
                                             name=f"pre{n}_{m}")
                            nc.scalar.activation(pre_sb[:], pm[:], AF.Silu,
                                                 scale=scal_t[:, m:m + 1])
                            xr_c = op.tile([P, FB], f32, tag="xrc", bufs=3,
                                           name=f"xrc{n}_{m}")
                            nc.sync.dma_start(
                                out=xr_c[:],
                                in_=I["xrow"][m * P:(m + 1) * P, n * FB:(n + 1) * FB])
                            nc.vector.scalar_tensor_tensor(
                                out=xr_c[:], in0=pre_sb[:], scalar=alpha_t[:],
                                in1=xr_c[:], op0=ALU.mult, op1=ALU.add)
                            nc.sync.dma_start(
                                out=out_ap[m * P:(m + 1) * P, n * FB:(n + 1) * FB],
                                in_=xr_c[:])
                        if n + 3 < NN:
                            wk_chunk(n + 3)


_CACHE = {}


def _build():
    if "nc" in _CACHE:
        return _CACHE["nc"]
    nc = bacc.Bacc("TRN2", target_bir_lowering=False, debug=False,
                   num_devices=NCORES)
    shapes = dict(
        xt=([8, P, 4 * SL], bf), xrow=([SL, D], f32), xrow_bf=([SL, D], bf),
        wk=([NN, 8, P, 4 * FB], bf), wk_aux=([NN, P, FB], bf),
        dwt=([16, P, 2 * KI], bf), xg=([4, P, D], bf), ag=([4, P, W], bf),
        entw=([NE, W, KD], bf), entt=([P, NE * W], bf), gwt=([P, KI], bf),
        uwt=([P, KI], bf), sst=([W, SL], bf), mask=([W, NE], f32),
        alpha_b=([P, 1], f32), aux_init=([P, SL], bf),
    )
    I = {name: nc.dram_tensor(name, shp, dt, kind="ExternalInput").ap()
         for name, (shp, dt) in shapes.items()}
    out_ap = nc.dram_tensor("out", [SL, D], f32, kind="ExternalOutput").ap()
    with tile.TileContext(nc) as tc:
        _kernel_body(nc, tc, I, out_ap)
    nc.compile()
    _CACHE["nc"] = nc
    return nc


def kernel(**inputs):
    nc = _build()
    in_maps = [build_core_inputs(inputs, c) for c in range(NCORES)]
    res = run_bass_kernel_spmd(nc, in_maps, core_ids=list(range(NCORES)))
    out = np.empty((B, S, D), np.float32)
    for c in range(NCORES):
        b, h = c // 2, c % 2
        out[b, h * SL:(h + 1) * SL] = res.results[c]["out"]
    return out


if __name__ == "__main__":
    rng = np.random.default_rng(0)
    inp = {
        "output_hidden_states": rng.standard_normal((B, S, D)).astype(np.float32),
        "words_ents": rng.integers(0, 100000, (B, W, E)).astype(np.int64),
        "words_subtoken": rng.integers(0, S, (B, W, T)).astype(np.int64),
        "input_ids": rng.integers(0, 32000, (B, S)).astype(np.int64),
        "concept_embed": (rng.standard_normal((100000, KD)) * 0.02).astype(np.float32),
        "sentinel": (rng.standard_normal((1, KD)) * 0.02).astype(np.float32),
        "ln_weight": np.ones(D, np.float32),
        "gate_w": (rng.standard_normal((KI, KD)) * 0.02).astype(np.float32),
        "up_w": (rng.standard_normal((KI, KD)) * 0.02).astype(np.float32),
        "down_w": (rng.standard_normal((D, KI)) * 0.02).astype(np.float32),
        "mlp_w": (rng.standard_normal((D, D + KD)) * 0.01).astype(np.float32),
        "mlp_b": np.zeros(D, np.float32),
        "alpha": np.array([0.5], np.float32),
    }
    out = kernel(**inp)
    print("kernel ran, out shape", out.shape, "mean", out.mean())



# revision 13
# speedup vs baseline: 1.5371x; 1.5371x over previous
"""Trainium2 Bass kernel for nn_AdapterMLP (gnn_message_passing).

Strategy (8 independent NeuronCores, no collectives):
  - Shard (batch=4) x (seq halves=2) -> 8 shards of [1024, 4096] rows.
  - All gather/scatter index structure is resolved on the host into
    dense one-hot matmul operands (A_g for the subtoken mean-pool
    "message passing" gather, S_sel for the last-wins scatter), so the
    device kernel is pure dense compute.
  - Algebraic shortcut: aw[w,e] = <ents_t[w,e,:], b[w,:]> is computed as
    <(g*u)[w,e,:], (b @ down_w)[w,:]>, eliminating the [1152,1024]x
    [1024,4096] per-item down-projection (8x fewer word-branch FLOPs).
  - The scatter branch is folded into the big MLP as one extra K-tile:
    pre = scale_s*(x @ Wh'^T) + [tmpT | 1] @ [Wt^T ; mlp_b], where
    Wh' = mlp_w[:, :D] * ln_weight (RMS scale commutes past the matmul).
  - TensorE runs bf16 (f32 accumulate in PSUM); weights/activations are
    pre-cast/pre-transposed on the host.
"""
import os
import sys

sys.path.insert(0, "/opt/trn_rl_repo")

import numpy as np
from ml_dtypes import bfloat16, float8_e4m3

import concourse.bass as bass
import concourse.bacc as bacc
import concourse.tile as tile
from concourse import mybir
from concourse.bass_utils import run_bass_kernel_spmd

B, S, D = 4, 2048, 4096
W, E, T = 128, 8, 4
KD, KI = 100, 1024
EPS = 1e-06
NCORES = 8
SL = S // 2        # 1024 rows per core
GR = 512           # gathered rows per core (W*T upper bound)
P = 128
FB = 512           # psum free dim
NK = D // P        # 32 k-tiles
NN = D // FB       # 8 n-chunks
NM = SL // P       # 8 m-tiles
NE = E + 1         # 9

f32 = mybir.dt.float32
bf = mybir.dt.bfloat16
f8 = mybir.dt.float8e4
AF = mybir.ActivationFunctionType
ALU = mybir.AluOpType
AX = mybir.AxisListType
PM = mybir.MatmulPerfMode

S_X = 16.0       # fp8 scale on activations (x)
S_W = 512.0      # fp8 scale on mlp weights
SC = S_X * S_W   # folded out via the per-row RMS scale


def _bf(a):
    return np.ascontiguousarray(a.astype(bfloat16))


def _f8(a):
    return np.ascontiguousarray(np.clip(a, -240.0, 240.0).astype(float8_e4m3))


def build_core_inputs(inp, core):
    """Host preprocessing for one core: slice/transpose/cast + index->matrix."""
    b, h = core // 2, core % 2
    r0 = h * SL
    x = np.asarray(inp["output_hidden_states"], np.float32)
    we_b = np.asarray(inp["words_ents"])[b]
    ws_b = np.asarray(inp["words_subtoken"])[b]
    ce = np.asarray(inp["concept_embed"], np.float32)
    sent = np.asarray(inp["sentinel"], np.float32).reshape(KD)
    lnw = np.asarray(inp["ln_weight"], np.float32)
    gw = np.asarray(inp["gate_w"], np.float32)
    uw = np.asarray(inp["up_w"], np.float32)
    dw = np.asarray(inp["down_w"], np.float32)
    mw = np.asarray(inp["mlp_w"], np.float32)
    mb = np.asarray(inp["mlp_b"], np.float32)
    alpha = float(np.asarray(inp["alpha"]).reshape(-1)[0])

    xl = x[b, r0:r0 + SL]                                    # [SL, D]
    xt = np.ascontiguousarray(xl.T)                          # [D, SL]

    # b-gather rows: unique subtoken indices of this item (pad index S dropped)
    idxf = np.where(ws_b == -1, S, ws_b)                     # [W,T]
    flat = idxf.reshape(-1)
    uniq = np.unique(flat[flat < S])
    gidx = np.zeros(GR, np.int64)
    gidx[:uniq.size] = uniq
    xg = x[b, gidx]                                          # [GR, D]
    cnt = np.maximum(np.sum(ws_b != -1, axis=1), 1).astype(np.float32)
    ag = np.zeros((GR, W), np.float32)
    pos = {int(s_): i for i, s_ in enumerate(uniq)}
    for w in range(W):
        for t in range(T):
            s_ = int(idxf[w, t])
            if s_ < S:
                ag[pos[s_], w] += 1.0 / cnt[w]

    # entity embeddings (host gather of the concept table)
    we_idx = np.where(we_b == -1, 0, we_b)
    ents = ce[we_idx]                                        # [W,E,KD]
    ent_ori = np.concatenate(
        [ents, np.broadcast_to(sent.reshape(1, 1, KD), (W, 1, KD))], axis=1)
    entw = np.ascontiguousarray(ent_ori.transpose(1, 0, 2))  # [NE, W, KD]
    entt = np.zeros((P, NE * W), np.float32)                 # KD padded to 128
    entt[:KD] = entw.reshape(NE * W, KD).T

    # scatter one-hot: winner = last (w,t) in flat order; local half only
    sst = np.zeros((W, SL), np.float32)
    winner = {}
    for w in range(W):
        for t in range(T):
            s_ = int(idxf[w, t])
            if s_ < S:
                winner[s_] = w
    for s_, w in winner.items():
        if r0 <= s_ < r0 + SL:
            sst[w, s_ - r0] = 1.0

    # weights: fold ln into Wh and down_w; pre-transpose; tile wk for DMA
    whT = (mw[:, :D] * lnw[None, :]).T                       # [D, D]
    wtT = mw[:, D:].T                                        # [KD, D]
    wk = np.zeros((NN, NK + 1, P, FB), np.float32)
    for n in range(NN):
        cs = slice(n * FB, (n + 1) * FB)
        for k in range(NK):
            wk[n, k] = whT[k * P:(k + 1) * P, cs]
        wk[n, NK, :KD] = wtT[:, cs]
        wk[n, NK, KD] = mb[cs]
    dwt = dw * lnw[:, None]                                  # [D, KI]

    mask = np.where(
        np.concatenate([we_b, np.ones((W, 1), we_b.dtype)], -1) == -1,
        -1e9, 0.0).astype(np.float32)

    aux_init = np.zeros((P, SL), np.float32)
    aux_init[KD] = 1.0

    # batch 4 k-tiles per DMA: [G, 128, 4*inner] contiguous blocks
    xt_big = xt.reshape(NK, P, SL).reshape(8, 4, P, SL).transpose(0, 2, 1, 3).reshape(8, P, 4 * SL)
    wk_big = wk[:, :NK].reshape(NN, 8, 4, P, FB).transpose(0, 1, 3, 2, 4).reshape(NN, 8, P, 4 * FB)
    wk_aux = np.ascontiguousarray(wk[:, NK])
    dwt_big = dwt.reshape(NK, P, KI).reshape(16, 2, P, KI).transpose(0, 2, 1, 3).reshape(16, P, 2 * KI)
    return dict(
        xt=_f8(xt_big * S_X),
        xrow=np.ascontiguousarray(xl),
        xrow_bf=_bf(xl),
        wk=_f8(wk_big * S_W),
        wk_aux=_bf(wk_aux),
        dwt=_bf(dwt_big),
        xg=_bf(xg).reshape(4, P, D),
        ag=_bf(ag).reshape(4, P, W),
        entw=_bf(entw),
        entt=_bf(entt),
        gwt=_bf(np.concatenate([gw.T, np.zeros((P - KD, KI), np.float32)], 0)),
        uwt=_bf(np.concatenate([uw.T, np.zeros((P - KD, KI), np.float32)], 0)),
        sst=_bf(sst),
        mask=np.ascontiguousarray(mask),
        alpha_b=np.full((P, 1), alpha, np.float32),
        aux_init=_bf(aux_init),
    )


def _kernel_body(nc, tc, I, out_ap):
    with tc.tile_pool(name="res", bufs=1) as res, \
         tc.tile_pool(name="small", bufs=1) as small:
        # ======== sync-queue DMAs in priority order ========
        xt_big = []
        for g in range(8):
            t = res.tile([P, 4 * SL], f8, tag=f"xt{g}")
            nc.sync.dma_start(out=t[:], in_=I["xt"][g])
            xt_big.append(t)

        def xt_pair(j, kk, m):
            # [P, 2, 128] stationary pair: k-tiles 4j+kk, 4j+kk+1
            return xt_big[j][:].rearrange(
                "p (k s) -> p k s", k=4)[:, kk:kk + 2, m * P:(m + 1) * P]

        with tc.tile_pool(name="wk0p", bufs=1) as wk0p, \
             tc.tile_pool(name="mpsum", bufs=1, space="PSUM") as mps:
            wp = tc.alloc_tile_pool(name="word", bufs=1)
            # word inputs (sync queue, right after xt)
            xg_tiles = []
            for g in range(4):
                xg_t = wp.tile([P, D], bf, tag=f"xg{g}", name=f"xg{g}")
                nc.sync.dma_start(out=xg_t[:], in_=I["xg"][g])
                xg_tiles.append(xg_t)
            agm = wp.tile([P, 4 * W], bf, tag="agm")
            for g in range(4):
                nc.sync.dma_start(out=agm[:, g * W:(g + 1) * W], in_=I["ag"][g])
            entt_t = wp.tile([P, NE * W], bf, tag="entt")
            nc.sync.dma_start(out=entt_t[:], in_=I["entt"][:])
            gwt_t = wp.tile([P, KI], bf, tag="gwt")
            nc.sync.dma_start(out=gwt_t[:], in_=I["gwt"][:])
            uwt_t = wp.tile([P, KI], bf, tag="uwt")
            nc.sync.dma_start(out=uwt_t[:], in_=I["uwt"][:])
            sst_t = wp.tile([P, SL], bf, tag="sst")
            nc.sync.dma_start(out=sst_t[:], in_=I["sst"][:])
            dwt_tiles = []
            for kb in range(16):
                dwt_t = wp.tile([P, 2 * KI], bf, tag=f"dwt{kb % 2}",
                                name=f"dwt{kb}")
                nc.sync.dma_start(out=dwt_t[:], in_=I["dwt"][kb])
                dwt_tiles.append(dwt_t)

            # var-pass input (sync queue, after dwt)
            mv = tc.alloc_tile_pool(name="mv", bufs=1)
            if True:
                xr_tiles = []
                for m in range(NM):
                    xr_t = mv.tile([P, D], bf, tag="xr", bufs=2, name=f"xr{m}")
                    nc.sync.dma_start(out=xr_t[:],
                                      in_=I["xrow_bf"][m * P:(m + 1) * P, :])
                    xr_tiles.append(xr_t)

                # ======== scalar-queue DMAs (small) ========
                aux_t = res.tile([P, SL], bf, tag="aux")
                nc.scalar.dma_start(out=aux_t[:], in_=I["aux_init"][:])
                alpha_t = small.tile([P, 1], f32, tag="alpha")
                nc.scalar.dma_start(out=alpha_t[:], in_=I["alpha_b"][:])
                mask_t = small.tile([P, NE], f32, tag="mask")
                nc.scalar.dma_start(out=mask_t[:], in_=I["mask"][:])
                ent_t = wp.tile([P, NE * KD], bf, tag="entw")
                for e in range(NE):
                    nc.scalar.dma_start(out=ent_t[:, e * KD:(e + 1) * KD],
                                        in_=I["entw"][e])

                # ======== gpsimd queue: wk chunk 0 ========
                wk_cache = {}
                grp0 = []
                for j in range(8):
                    wt = wk0p.tile([P, 4 * FB], f8, tag=f"wk0g{j}", name=f"wk0g{j}")
                    nc.gpsimd.dma_start(out=wt[:], in_=I["wk"][0, j])
                    grp0.append(wt)
                at0 = wk0p.tile([P, FB], bf, tag="wk0aux")
                nc.gpsimd.dma_start(out=at0[:], in_=I["wk_aux"][0])
                wk_cache[0] = (grp0, at0)

                def wk_pair(wt, kk):
                    # [P, 2, FB] moving pair matching xt_pair's k-tiles
                    return wt[:].rearrange("p (k f) -> p k f", k=4)[:, kk:kk + 2, :]

                scal_t = small.tile([P, NM], f32, tag="scal")
                eps_t = small.tile([P, 1], f32, tag="eps")
                nc.vector.memset(eps_t[:], EPS)
                eps_sc = small.tile([P, 1], f32, tag="eps_sc")
                nc.vector.memset(eps_sc[:], EPS * SC * SC)

                # ======== word branch compute ========
                pms03 = []

                def emit_pm0(m):
                    pm = mps.tile([P, FB], f32, tag="pm", bufs=6,
                                  name=f"pm0_{m}")
                    for j in range(8):
                        for kk in (0, 2):
                            nc.tensor.matmul(
                                pm[:], lhsT=xt_pair(j, kk, m),
                                rhs=wk_pair(grp0[j], kk),
                                start=(j == 0 and kk == 0), stop=False,
                                perf_mode=PM.DoubleRow)
                    pms03.append(pm)

                with tc.tile_pool(name="wpsum", bufs=2, space="PSUM") as wps:
                    # ACT: squares of gathered rows -> rc (gates ags on DVE)
                    rcs = []
                    for g in range(4):
                        sq = mv.tile([P, D], bf, tag="sqscr", bufs=1, name=f"sqw{g}")
                        var = small.tile([P, 1], f32, tag=f"varg{g}")
                        nc.scalar.activation(sq[:], xg_tiles[g][:], AF.Square,
                                             accum_out=var[:])
                        sd = small.tile([P, 1], f32, tag=f"sdg{g}")
                        nc.scalar.activation(sd[:], var[:], AF.Sqrt, bias=eps_t[:],
                                             scale=1.0 / D)
                        rc = small.tile([P, 1], f32, tag=f"rcg{g}")
                        nc.vector.reciprocal(rc[:], sd[:])
                        rcs.append(rc)
                    ags = wp.tile([P, 4 * W], bf, tag="ags")
                    for g in range(4):
                        nc.vector.tensor_scalar_mul(ags[:, g * W:(g + 1) * W],
                                                    agm[:, g * W:(g + 1) * W],
                                                    rcs[g][:])

                    # DVE: var of main rows (stt with accum), before gu chain
                    varms = []
                    for m in range(NM):
                        vscr = mv.tile([P, D], bf, tag="sqscr", bufs=1, name=f"vscr{m}")
                        varm = small.tile([P, 1], f32, tag=f"varm{m}")
                        nc.vector.scalar_tensor_tensor(
                            out=vscr[:], in0=xr_tiles[m][:], scalar=1.0,
                            in1=xr_tiles[m][:], op0=ALU.mult, op1=ALU.mult,
                            accum_out=varm[:])
                        varms.append(varm)
                    mv.release()

                    # PE: bT
                    bt_tiles = []
                    for dk in range(NK):
                        ps = wps.tile([P, W], f32, tag="wps", name=f"btps{dk}")
                        for g in range(4):
                            nc.tensor.matmul(ps[:],
                                             lhsT=xg_tiles[g][:, dk * P:(dk + 1) * P],
                                             rhs=ags[:, g * W:(g + 1) * W],
                                             start=(g == 0), stop=(g == 3))
                        bt = wp.tile([P, W], bf, tag=f"bt{dk}", name=f"bt{dk}")
                        nc.scalar.copy(bt[:], ps[:])
                        bt_tiles.append(bt)

                    emit_pm0(0)

                    # PE: c = b @ (down_w * lnw)
                    c_bf = wp.tile([P, KI], bf, tag="c")
                    cps = []
                    for i in range(2):
                        cpsi = wps.tile([P, FB], f32, tag="wps", name=f"c_ps{i}")
                        cps.append(cpsi)
                    for kb in range(16):
                        for kk in range(2):
                            dk = kb * 2 + kk
                            for i in range(2):
                                nc.tensor.matmul(
                                    cps[i][:], lhsT=bt_tiles[dk][:],
                                    rhs=dwt_tiles[kb][:, kk * KI + i * FB:kk * KI + (i + 1) * FB],
                                    start=(dk == 0), stop=(dk == NK - 1))
                    for i in range(2):
                        nc.scalar.copy(c_bf[:, i * FB:(i + 1) * FB], cps[i][:])

                    emit_pm0(1)
                    emit_pm0(2)

                    # PE/ACT/DVE: gate/up + gu; aw via stt-accum
                    aw_t = small.tile([P, NE], f32, tag="aw")
                    for e in range(NE):
                        g_sb = wp.tile([P, KI], bf, tag="gsb", bufs=1)
                        u_sb = wp.tile([P, KI], bf, tag="usb", bufs=1)
                        for i in range(2):
                            gp = wps.tile([P, FB], f32, tag="wps", name=f"gp{e}_{i}")
                            nc.tensor.matmul(gp[:], lhsT=entt_t[:, e * P:(e + 1) * P],
                                             rhs=gwt_t[:, i * FB:(i + 1) * FB],
                                             start=True, stop=True)
                            nc.scalar.activation(g_sb[:, i * FB:(i + 1) * FB], gp[:],
                                                 AF.Silu)
                            up = wps.tile([P, FB], f32, tag="wps", name=f"up{e}_{i}")
                            nc.tensor.matmul(up[:], lhsT=entt_t[:, e * P:(e + 1) * P],
                                             rhs=uwt_t[:, i * FB:(i + 1) * FB],
                                             start=True, stop=True)
                            nc.scalar.copy(u_sb[:, i * FB:(i + 1) * FB], up[:])
                        gu = wp.tile([P, KI], bf, tag="gu", bufs=2)
                        nc.vector.tensor_mul(gu[:], g_sb[:], u_sb[:])
                        scr = wp.tile([P, KI], bf, tag="awscr", bufs=1)
                        nc.vector.scalar_tensor_tensor(
                            out=scr[:], in0=gu[:], scalar=1.0, in1=c_bf[:],
                            op0=ALU.mult, op1=ALU.mult,
                            accum_out=aw_t[:, e:e + 1])
                        if e == 4:
                            emit_pm0(3)

                    # ACT: finish per-row scales (before tps copies)
                    sdall_t = small.tile([P, NM], f32, tag="sdall")
                    for m in range(NM):
                        nc.scalar.activation(sdall_t[:, m:m + 1], varms[m][:],
                                         AF.Sqrt, bias=eps_sc[:],
                                         scale=SC * SC / D)
                        nc.vector.reciprocal(scal_t[:, m:m + 1],
                                         sdall_t[:, m:m + 1])

                    # binv[kd, s] = sd[s]; aux /= scal  (so one psum + ACT-scale works)
                    sd_d = nc.dram_tensor("sd_rt", [P, NM], f32, kind="Internal").ap()
                    nc.scalar.dma_start(out=sd_d[:], in_=sdall_t[:])
                    sd_ff = small.tile([1, SL], f32, tag="sd_ff")
                    nc.scalar.dma_start(
                        out=sd_ff[:].rearrange("a (m p) -> a m p", m=NM),
                        in_=sd_d.rearrange("p m -> m p")[None])
                    sd_fb = small.tile([1, SL], bf, tag="sd_fb")
                    nc.vector.tensor_copy(sd_fb[:], sd_ff[:])
                    ones_r = small.tile([1, P], bf, tag="ones_r")
                    nc.vector.memset(ones_r[:], 1.0)
                    binv = res.tile([P, SL], bf, tag="binv")
                    for i in range(SL // FB):
                        bp = mps.tile([P, FB], f32, tag="pm", bufs=6, name=f"binvp{i}")
                        nc.tensor.matmul(bp[:], lhsT=ones_r[:],
                                                 rhs=sd_fb[:, i * FB:(i + 1) * FB],
                                                 start=True, stop=True)
                        nc.scalar.copy(binv[:, i * FB:(i + 1) * FB], bp[:])

                    if True:
                        # DVE: softmax + attn chain
                        awm = small.tile([P, NE], f32, tag="awm")
                        nc.vector.tensor_add(awm[:], aw_t[:], mask_t[:])
                        mx = small.tile([P, 1], f32, tag="mx")
                        nc.vector.reduce_max(mx[:], awm[:], axis=AX.X)
                        nmx = small.tile([P, 1], f32, tag="nmx")
                        nc.vector.tensor_scalar_mul(nmx[:], mx[:], -1.0)
                        expt = small.tile([P, NE], f32, tag="expt")
                        sume = small.tile([P, 1], f32, tag="sume")
                        nc.scalar.activation(expt[:], awm[:], AF.Exp, bias=nmx[:],
                                             accum_out=sume[:])
                        rse = small.tile([P, 1], f32, tag="rse")
                        nc.vector.reciprocal(rse[:], sume[:])
                        attn = small.tile([P, NE], f32, tag="attn")
                        nc.vector.tensor_scalar_mul(attn[:], expt[:], rse[:])
                        acc_prev = wp.tile([P, KD], f32, tag="acc", bufs=2)
                        nc.vector.tensor_scalar_mul(acc_prev[:], ent_t[:, 0:KD],
                                                    attn[:, 0:1])
                        for e in range(1, NE):
                            acc_new = wp.tile([P, KD], f32, tag="acc", bufs=2,
                                              name=f"acc{e}")
                            nc.vector.scalar_tensor_tensor(
                                out=acc_new[:], in0=ent_t[:, e * KD:(e + 1) * KD],
                                scalar=attn[:, e:e + 1], in1=acc_prev[:],
                                op0=ALU.mult, op1=ALU.add)
                            acc_prev = acc_new
                        ao_pad = wp.tile([P, P], bf, tag="ao_pad")
                        nc.vector.memset(ao_pad[:], 0.0)
                        nc.scalar.copy(ao_pad[:, 0:KD], acc_prev[:])

                        # PE: scatter matmul into aux k-tile
                        for i in range(SL // FB):
                            tps = wps.tile([P, FB], f32, tag="wps", name=f"tps{i}")
                            nc.tensor.matmul(tps[:], lhsT=ao_pad[:],
                                             rhs=sst_t[:, i * FB:(i + 1) * FB],
                                             start=True, stop=True)
                            nc.scalar.copy(aux_t[0:KD, i * FB:(i + 1) * FB],
                                           tps[0:KD, :])
                        nc.vector.tensor_mul(aux_t[:], aux_t[:], binv[:])

            if os.environ.get("K_PROBE"):
                dbg_scal = nc.dram_tensor("dbg_scal", [P, NM], f32, kind="Internal").ap()
                nc.sync.dma_start(out=dbg_scal[:], in_=scal_t[:])
                dbg_aux = nc.dram_tensor("dbg_aux", [P, SL], bf, kind="Internal").ap()
                nc.sync.dma_start(out=dbg_aux[:], in_=aux_t[:])
                dbg_aw = nc.dram_tensor("dbg_aw", [P, NE], f32, kind="Internal").ap()
                nc.sync.dma_start(out=dbg_aw[:], in_=aw_t[:])
                dbg_c = nc.dram_tensor("dbg_c", [P, KI], bf, kind="Internal").ap()
                nc.sync.dma_start(out=dbg_c[:], in_=c_bf[:])

            # ---- word pool closed; epilogue + main loop ----
            wp.release()
            with tc.tile_pool(name="op", bufs=2) as op:
                # ======== epilogue helper ========
                def emit_tail(n, m, pm, wk_aux_t):
                    pa = mps.tile([P, FB], f32, tag="pa", bufs=2,
                                  name=f"pa{n}_{m}")
                    nc.tensor.matmul(pa[:], lhsT=aux_t[:, m * P:(m + 1) * P],
                                     rhs=wk_aux_t[:], start=True, stop=True)
                    pa_sb = op.tile([P, FB], f32, tag="pasb", bufs=2,
                                    name=f"pasb{n}_{m}")
                    nc.scalar.copy(pa_sb[:], pa[:])
                    pre = op.tile([P, FB], f32, tag="pre", bufs=2,
                                  name=f"pre{n}_{m}")
                    nc.vector.scalar_tensor_tensor(
                        out=pre[:], in0=pm[:], scalar=scal_t[:, m:m + 1],
                        in1=pa_sb[:], op0=ALU.mult, op1=ALU.add)
                    nc.scalar.activation(pre[:], pre[:], AF.Silu)
                    xr_c = op.tile([P, FB], f32, tag="xrc", bufs=2,
                                   name=f"xrc{n}_{m}")
                    nc.sync.dma_start(
                        out=xr_c[:],
                        in_=I["xrow"][m * P:(m + 1) * P, n * FB:(n + 1) * FB])
                    nc.vector.scalar_tensor_tensor(
                        out=xr_c[:], in0=pre[:], scalar=alpha_t[:],
                        in1=xr_c[:], op0=ALU.mult, op1=ALU.add)
                    nc.sync.dma_start(
                        out=out_ap[m * P:(m + 1) * P, n * FB:(n + 1) * FB],
                        in_=xr_c[:])

                # ======== main loop ========
                with tc.tile_pool(name="wkp", bufs=1) as wkp:
                    def wk_chunk(n):
                        if n in wk_cache:
                            return wk_cache[n]
                        grp = []
                        for j in range(8):
                            wt = wkp.tile([P, 4 * FB], f8, tag=f"wkg{j}",
                                          bufs=2, name=f"wk{n}g{j}")
                            nc.gpsimd.dma_start(out=wt[:], in_=I["wk"][n, j])
                            grp.append(wt)
                        at = wkp.tile([P, FB], bf, tag="wk_aux", bufs=2,
                                      name=f"wka{n}")
                        nc.gpsimd.dma_start(out=at[:], in_=I["wk_aux"][n])
                        wk_cache[n] = (grp, at)
                        return wk_cache[n]

                    wk_chunk(1)
                    wk_chunk(2)
                    for n in range(NN):
                        wk_grp, wk_aux_t = wk_chunk(n)
                        for m in range(NM):
                            if n == 0 and m < 4:
                                pm = pms03[m]
                            else:
                                pm = mps.tile([P, FB], f32, tag="pm",
                                              bufs=6, name=f"pm{n}_{m}")
                                for j in range(8):
                                    for kk in (0, 2):
                                        nc.tensor.matmul(
                                            pm[:], lhsT=xt_pair(j, kk, m),
                                            rhs=wk_pair(wk_grp[j], kk),
                                            start=(j == 0 and kk == 0),
                                            stop=False,
                                            perf_mode=PM.DoubleRow)
                            nc.tensor.matmul(pm[:], lhsT=aux_t[:, m * P:(m + 1) * P],
                                             rhs=wk_aux_t[:], start=False, stop=True)
                            pre_sb = op.tile([P, FB], f32, tag="pre", bufs=3,
                                             name=f"pre{n}_{m}")
                            nc.scalar.activation(pre_sb[:], pm[:], AF.Silu,
                                                 scale=scal_t[:, m:m + 1])
                            xr_c = op.tile([P, FB], f32, tag="xrc", bufs=3,
                                           name=f"xrc{n}_{m}")
                            nc.sync.dma_start(
                                out=xr_c[:],
                                in_=I["xrow"][m * P:(m + 1) * P, n * FB:(n + 1) * FB])
                            nc.vector.scalar_tensor_tensor(
                                out=xr_c[:], in0=pre_sb[:], scalar=alpha_t[:],
                                in1=xr_c[:], op0=ALU.mult, op1=ALU.add)
                            nc.sync.dma_start(
                                out=out_ap[m * P:(m + 1) * P, n * FB:(n + 1) * FB],
                                in_=xr_c[:])
                        if n + 3 < NN:
                            wk_chunk(n + 3)


_CACHE = {}


def _build():
    if "nc" in _CACHE:
        return _CACHE["nc"]
    nc = bacc.Bacc("TRN2", target_bir_lowering=False, debug=False,
                   num_devices=NCORES)
    shapes = dict(
        xt=([8, P, 4 * SL], f8), xrow=([SL, D], f32), xrow_bf=([SL, D], bf),
        wk=([NN, 8, P, 4 * FB], f8), wk_aux=([NN, P, FB], bf),
        dwt=([16, P, 2 * KI], bf), xg=([4, P, D], bf), ag=([4, P, W], bf),
        entw=([NE, W, KD], bf), entt=([P, NE * W], bf), gwt=([P, KI], bf),
        uwt=([P, KI], bf), sst=([W, SL], bf), mask=([W, NE], f32),
        alpha_b=([P, 1], f32), aux_init=([P, SL], bf),
    )
    I = {name: nc.dram_tensor(name, shp, dt, kind="ExternalInput").ap()
         for name, (shp, dt) in shapes.items()}
    out_ap = nc.dram_tensor("out", [SL, D], f32, kind="ExternalOutput").ap()
    with tile.TileContext(nc) as tc:
        _kernel_body(nc, tc, I, out_ap)
    nc.compile()
    _CACHE["nc"] = nc
    return nc


def kernel(**inputs):
    nc = _build()
    in_maps = [build_core_inputs(inputs, c) for c in range(NCORES)]
    res = run_bass_kernel_spmd(nc, in_maps, core_ids=list(range(NCORES)))
    out = np.empty((B, S, D), np.float32)
    for c in range(NCORES):
        b, h = c // 2, c % 2
        out[b, h * SL:(h + 1) * SL] = res.results[c]["out"]
    return out


if __name__ == "__main__":
    rng = np.random.default_rng(0)
    inp = {
        "output_hidden_states": rng.standard_normal((B, S, D)).astype(np.float32),
        "words_ents": rng.integers(0, 100000, (B, W, E)).astype(np.int64),
        "words_subtoken": rng.integers(0, S, (B, W, T)).astype(np.int64),
        "input_ids": rng.integers(0, 32000, (B, S)).astype(np.int64),
        "concept_embed": (rng.standard_normal((100000, KD)) * 0.02).astype(np.float32),
        "sentinel": (rng.standard_normal((1, KD)) * 0.02).astype(np.float32),
        "ln_weight": np.ones(D, np.float32),
        "gate_w": (rng.standard_normal((KI, KD)) * 0.02).astype(np.float32),
        "up_w": (rng.standard_normal((KI, KD)) * 0.02).astype(np.float32),
        "down_w": (rng.standard_normal((D, KI)) * 0.02).astype(np.float32),
        "mlp_w": (rng.standard_normal((D, D + KD)) * 0.01).astype(np.float32),
        "mlp_b": np.zeros(D, np.float32),
        "alpha": np.array([0.5], np.float32),
    }
    out = kernel(**inp)
    print("kernel ran, out shape", out.shape, "mean", out.mean())

